# revision 20
# baseline (speedup 1.0000x reference)
"""Causal multi-head attention (B=2, L=1024, P=4, HID=1024, NH=16, HS=64)
with RoPE, distributed data-parallel over the 8 (b, p) shards across 8
TRN2 NeuronCores. Self-contained: kernel(**inputs) -> np.ndarray."""

import numpy as np
import ml_dtypes

import concourse.bacc as bacc
import concourse.mybir as mybir
import concourse.tile as tile
from concourse.bass_utils import run_bass_kernel_spmd

B, L, P, HID = 2, 1024, 4, 1024
NH, HS = 16, 64
NCORES = 8
KT = 8          # contraction tiles of 128 over HID
DT = 8          # d-tiles of 128 (2 heads each)
NTT = 2         # token tiles of 512
BF16 = mybir.dt.bfloat16
F32 = mybir.dt.float32
AF = mybir.ActivationFunctionType
ALU = mybir.AluOpType
NPBF16 = ml_dtypes.bfloat16

def build_nc(debug=False):
    nc = bacc.Bacc()
    xT = nc.declare_dram_parameter("xT", [HID, L], BF16, isOutput=False)
    wqT = nc.declare_dram_parameter("wqT", [HID, HID], BF16, isOutput=False)
    wkT = nc.declare_dram_parameter("wkT", [HID, HID], BF16, isOutput=False)
    wvT = nc.declare_dram_parameter("wvT", [HID, HID], BF16, isOutput=False)
    owT = nc.declare_dram_parameter("owT", [HID, HID], BF16, isOutput=False)
    cosT2 = nc.declare_dram_parameter("cosT2", [128, L], BF16, isOutput=False)
    sinT2 = nc.declare_dram_parameter("sinT2", [128, L], BF16, isOutput=False)
    rmat = nc.declare_dram_parameter("rmat", [128, 128], BF16, isOutput=False)
    atri = nc.declare_dram_parameter("atri", [128, 128], BF16, isOutput=False)
    ident = nc.declare_dram_parameter("ident", [128, 128], BF16, isOutput=False)
    niota2 = nc.declare_dram_parameter("niota2", [128, 2, 128], F32, isOutput=False)
    mlen = nc.declare_dram_parameter("mlen", [128, 1], F32, isOutput=False)
    outT = nc.declare_dram_parameter("outT", [HID, L], F32, isOutput=True)
    dbg = {}
    if debug:
        dbg["qrope"] = nc.declare_dram_parameter("dbg_qrope", [128, DT, L], BF16, isOutput=True)
        dbg["krope"] = nc.declare_dram_parameter("dbg_krope", [128, DT, L], BF16, isOutput=True)
        dbg["v"] = nc.declare_dram_parameter("dbg_v", [128, 8, NH, 65], BF16, isOutput=True)
        dbg["o"] = nc.declare_dram_parameter("dbg_o", [128, DT, L], BF16, isOutput=True)
        dbg["sc"] = nc.declare_dram_parameter("dbg_sc", [128, 2, 512], F32, isOutput=True)
        dbg["es"] = nc.declare_dram_parameter("dbg_es", [128, 2, 512], BF16, isOutput=True)
        dbg["zb"] = nc.declare_dram_parameter("dbg_zb", [64, 512], F32, isOutput=True)
        dbg["rz"] = nc.declare_dram_parameter("dbg_rz", [64, 512], F32, isOutput=True)
        dbg["otraw"] = nc.declare_dram_parameter("dbg_otraw", [65, 512], F32, isOutput=True)

    with tile.TileContext(nc) as tc:
        with tc.tile_pool(name="consts", bufs=1) as cpool:
            xs = cpool.tile([128, KT, L], BF16, name="xs")
            nc.sync.dma_start(out=xs[:], in_=xT.rearrange("(k p) t -> p k t", p=128))
            wqs = cpool.tile([128, KT, HID], BF16, name="wqs")
            nc.sync.dma_start(out=wqs[:], in_=wqT.rearrange("(k p) d -> p k d", p=128))
            wks = cpool.tile([128, KT, HID], BF16, name="wks")
            nc.sync.dma_start(out=wks[:], in_=wkT.rearrange("(k p) d -> p k d", p=128))
            wvs = cpool.tile([128, KT, HID], BF16, name="wvs")
            nc.sync.dma_start(out=wvs[:], in_=wvT.rearrange("(k p) d -> p k d", p=128))
            ows = cpool.tile([128, KT, HID], BF16, name="ows")
            nc.sync.dma_start(out=ows[:], in_=owT.rearrange("(k p) h -> p k h", p=128))
            coss = cpool.tile([128, L], BF16, name="coss")
            nc.sync.dma_start(out=coss[:], in_=cosT2[:])
            sins = cpool.tile([128, L], BF16, name="sins")
            nc.sync.dma_start(out=sins[:], in_=sinT2[:])
            rms = cpool.tile([128, 128], BF16, name="rms")
            nc.sync.dma_start(out=rms[:], in_=rmat[:])
            ats = cpool.tile([128, 128], BF16, name="ats")
            nc.sync.dma_start(out=ats[:], in_=atri[:])
            ids = cpool.tile([128, 128], BF16, name="ids")
            nc.sync.dma_start(out=ids[:], in_=ident[:])
            nio = cpool.tile([128, 2, 128], F32, name="nio")
            nc.sync.dma_start(out=nio[:], in_=niota2[:])
            mls = cpool.tile([128, 1], F32, name="mls")
            nc.sync.dma_start(out=mls[:], in_=mlen[:])

            with tc.tile_pool(name="persist", bufs=1) as ppool:
                qrope = ppool.tile([128, DT, L], BF16, name="qrope")
                krope = ppool.tile([128, DT, L], BF16, name="krope")
                # v in natural layout [tok%128, tok//128, head, hs+ones]
                vs = ppool.tile([128, 8, NH, 128], BF16, name="vs")
                # normalized attention output, transposed: [d%128, head-pair, tok]
                os2 = ppool.tile([128, DT, L], BF16, name="os2")

                nc.gpsimd.memset(vs[:, :, :, 64:128], 0.0)
                nc.gpsimd.memset(vs[:, :, :, 64:65], 1.0)
                ones64 = ppool.tile([65, 64], BF16, name="ones64")
                nc.gpsimd.memset(ones64[:], 1.0)

                # ---------------- Phase 1: projections + rope -------------
                with (
                    tc.tile_pool(name="psum", bufs=1, space="PSUM") as pp,
                    tc.tile_pool(name="work", bufs=1) as wp1,
                ):
                    wp2 = wp1
                    pp1 = pp2 = pp3 = pp
                    # v = x @ wv^T in natural layout
                    for bt in range(8):          # token tile of 128
                        for dh in range(NTT):    # d half of 512 (8 heads)
                            vp = pp1.tile([128, 512], F32, name="vp", tag="A", bufs=2)
                            for k in range(KT):
                                nc.tensor.matmul(
                                    vp[:],
                                    xs[:, k, bt * 128:(bt + 1) * 128],
                                    wvs[:, k, dh * 512:(dh + 1) * 512],
                                    start=(k == 0), stop=(k == KT - 1),
                                )
                            nc.scalar.copy(
                                out=vs[:, bt, dh * 8:(dh + 1) * 8, 0:64],
                                in_=vp[:].rearrange("p (h e) -> p h e", e=64),
                            )
                    # per head-pair group: qk proj + rope, then attention.
                    # The Z-bcast/recip/norm tail of group g is deferred until
                    # after group g+1's projection matmuls are queued, so the
                    # in-order PE never waits on it.
                    def emit_norm(items):
                        for (g_, c_, h_, stge_) in items:
                            zbp = pp2.tile([64, 512], F32, name="zbp",
                                           tag="ot0" if h_ == 0 else "ot1", bufs=1)
                            nc.tensor.matmul(zbp[:], ones64[64:65, :], stge_[64:65, :],
                                             start=True, stop=True)
                            rz = wp2.tile([64, 512], F32, name="rz", tag=f"rz{h_}", bufs=2)
                            nc.vector.reciprocal_approx_fast(out=rz[:], in_=zbp[:])
                            if debug and g_ == 0 and c_ == 0 and h_ == 0:
                                nc.sync.dma_start(out=dbg["rz"][:], in_=rz[:])
                            if h_ == 0:
                                nc.vector.tensor_mul(
                                    os2[0:64, g_, c_ * 512:(c_ + 1) * 512],
                                    stge_[0:64, :],
                                    rz[:],
                                )
                            else:
                                stg = wp2.tile([64, 512], BF16, name="stg", tag="stg", bufs=2)
                                nc.vector.tensor_mul(stg[:], stge_[0:64, :], rz[:])
                                nc.sync.dma_start(
                                    out=os2[64:128, g_, c_ * 512:(c_ + 1) * 512],
                                    in_=stg[:],
                                )

                    pending = []
                    for kd in range(DT):
                        for t in range(NTT):
                            for (ws, dest) in ((wqs, qrope), (wks, krope)):
                                qp = pp1.tile([128, 512], F32, name="qp", tag="qp", bufs=1)
                                for k in range(KT):
                                    nc.tensor.matmul(
                                        qp[:],
                                        ws[:, k, kd * 128:(kd + 1) * 128],
                                        xs[:, k, t * 512:(t + 1) * 512],
                                        start=(k == 0), stop=(k == KT - 1),
                                    )
                                qraw = wp1.tile([128, 512], BF16, name="qraw", tag="qraw", bufs=3)
                                nc.vector.tensor_copy(qraw[:], qp[:])
                                rp = pp1.tile([128, 512], F32, name="rp", tag="rp", bufs=1)
                                nc.tensor.matmul(rp[:], rms[:], qraw[:], start=True, stop=True)
                                m1 = wp1.tile([128, 512], BF16, name="m1", tag="m1", bufs=3)
                                nc.gpsimd.tensor_mul(m1[:], qraw[:], coss[:, t * 512:(t + 1) * 512])
                                m2 = wp1.tile([128, 512], BF16, name="m2", tag="m2", bufs=3)
                                nc.vector.tensor_mul(m2[:], rp[:], sins[:, t * 512:(t + 1) * 512])
                                nc.gpsimd.tensor_add(
                                    dest[:, kd, t * 512:(t + 1) * 512], m1[:], m2[:]
                                )

                        # deferred norm tail of the previous group
                        emit_norm(pending)
                        pending = []

                        # ---- attention for this head pair ----
                        g = kd
                        for c in range(NTT):     # i tile of 512
                            ot = [
                                pp2.tile([128, 512], F32, name=f"ot{h}_{g}_{c}",
                                         tag="ot0" if h == 0 else "ot1", bufs=1)
                                for h in range(2)
                            ]
                            nb = 4 * c + 4

                            def emit_omm(b_, es_, off_):
                                for h in range(2):
                                    nc.tensor.matmul(
                                        ot[h][0:128, off_:512],
                                        vs[:, b_, 2 * g + h, :],
                                        es_[:, h, off_:],
                                        start=(b_ == 0), stop=(b_ == nb - 1),
                                    )

                            prev_omm = None
                            for b in range(nb):  # j block of 128
                                scp = pp2.tile([128, 2, 512], F32, name="scp", tag="A", bufs=2)
                                partial = b >= 4 * c
                                off = max(0, 128 * b - 512 * c)
                                for h in range(2):
                                    nc.tensor.matmul(
                                        scp[:, h, off:],
                                        krope[64 * h:64 * h + 64, g, b * 128:(b + 1) * 128],
                                        qrope[64 * h:64 * h + 64, g, c * 512 + off:(c + 1) * 512],
                                        start=True, stop=not partial,
                                        tile_position=(64 * h, 0),
                                    )
                                if partial:
                                    for h in range(2):
                                        nc.tensor.matmul(
                                            scp[:, h, off:off + 128],
                                            ats[:], ids[:],
                                            start=False, stop=True,
                                        )
                                es = wp2.tile([128, 2, 512], BF16, name="es", tag="es", bufs=3)
                                nc.scalar.activation(
                                    out=es[:, :, off:], in_=scp[:, :, off:],
                                    func=AF.Exp, scale=0.125,
                                )
                                if debug and g == 0 and c == 0 and b == 0:
                                    dsc = ppool.tile([128, 2, 512], F32, name="dsc")
                                    nc.scalar.copy(out=dsc[:], in_=scp[:])
                                    nc.sync.dma_start(out=dbg["sc"][:], in_=dsc[:])
                                    nc.sync.dma_start(out=dbg["es"][:], in_=es[:])
                                if prev_omm is not None:
                                    emit_omm(*prev_omm)
                                prev_omm = (b, es, off)
                            emit_omm(*prev_omm)
                            # drain ot -> SBUF right away (releases the PSUM bank)
                            for h in range(2):
                                stge = wp2.tile([65, 512], BF16, name="stge", tag=f"stge{h}", bufs=2)
                                nc.vector.tensor_copy(stge[:], ot[h][0:65, :])
                                if debug and g == 0 and c == 0 and h == 0:
                                    dot = ppool.tile([65, 512], F32, name="dot")
                                    nc.scalar.copy(out=dot[:], in_=ot[0][0:65, :])
                                    nc.sync.dma_start(out=dbg["otraw"][:], in_=dot[:])
                                pending.append((g, c, h, stge))
                    emit_norm(pending)

                    # ------------- Phase 3: output projection -------------
                    wp3 = wp1
                    outr = outT.rearrange("(k p) t -> p k t", p=128)
                    for kh in range(DT):
                        for t in range(NTT):
                            fp = pp3.tile([128, 512], F32, name="fp", tag="A", bufs=2)
                            for k in range(KT):
                                nc.tensor.matmul(
                                    fp[:],
                                    ows[:, k, kh * 128:(kh + 1) * 128],
                                    os2[:, k, t * 512:(t + 1) * 512],
                                    start=(k == 0), stop=(k == KT - 1),
                                )
                            fo = wp3.tile([128, 512], F32, name="fo", tag="fo", bufs=2)
                            nc.scalar.copy(out=fo[:], in_=fp[:])
                            nc.sync.dma_start(
                                out=outr[:, kh, t * 512:(t + 1) * 512], in_=fo[:]
                            )

                if debug:
                    nc.sync.dma_start(out=dbg["qrope"][:], in_=qrope[:])
                    nc.sync.dma_start(out=dbg["krope"][:], in_=krope[:])
                    nc.sync.dma_start(out=dbg["v"][:], in_=vs[:])
                    nc.sync.dma_start(out=dbg["o"][:], in_=os2[:])
    nc.finalize()
    return nc


def _host_consts():
    rmat = np.zeros((128, 128), np.float32)
    for m in range(128):
        if (m % 64) < 32:
            rmat[m + 32, m] = -1.0
        else:
            rmat[m - 32, m] = 1.0
    atri = -240.0 * np.triu(np.ones((128, 128), np.float32), 1)
    ident = np.eye(128, dtype=np.float32)
    niota2 = np.broadcast_to(-np.arange(128, dtype=np.float32), (128, 2, 128)).copy()
    mlen = (1.0 - np.arange(128, dtype=np.float32)).reshape(128, 1)
    return (rmat.astype(NPBF16), atri.astype(NPBF16), ident.astype(NPBF16),
            niota2, mlen)


def kernel(x, qw, kw, vw, ow, cos, sin, debug=False):
    x = np.asarray(x, np.float32)
    qw = np.asarray(qw, np.float32)
    kw = np.asarray(kw, np.float32)
    vw = np.asarray(vw, np.float32)
    ow = np.asarray(ow, np.float32)
    cos = np.asarray(cos, np.float32)
    sin = np.asarray(sin, np.float32)

    wqT = np.ascontiguousarray(qw.T).astype(NPBF16)
    wkT = np.ascontiguousarray(kw.T).astype(NPBF16)
    wvT = np.ascontiguousarray(vw.T).astype(NPBF16)
    owT = np.ascontiguousarray(ow.T).astype(NPBF16)
    cosT2 = np.concatenate([cos.T, cos.T], 0).astype(NPBF16)
    sinT2 = np.concatenate([sin.T, sin.T], 0).astype(NPBF16)
    rmat, atri, ident, niota2, mlen = _host_consts()

    shared = {
        "wqT": wqT, "wkT": wkT, "wvT": wvT, "owT": owT,
        "cosT2": np.ascontiguousarray(cosT2), "sinT2": np.ascontiguousarray(sinT2),
        "rmat": rmat, "atri": atri, "ident": ident,
        "niota2": niota2, "mlen": mlen,
    }
    in_maps = []
    for c in range(NCORES):
        b, p = divmod(c, P)
        xTc = np.ascontiguousarray(x[b, :, p, :].T).astype(NPBF16)
        m = dict(shared)
        m["xT"] = xTc
        in_maps.append(m)

    nc = build_nc(debug=debug)
    res = run_bass_kernel_spmd(nc, in_maps, list(range(NCORES)))

    y = np.empty((B, L, P, HID), np.float32)
    for c in range(NCORES):
        b, p = divmod(c, P)
        y[b, :, p, :] = res.results[c]["outT"].T
    if debug:
        return y, res
    return y


# revision 21
# speedup vs baseline: 1.0302x; 1.0302x over previous
"""Causal multi-head attention (B=2, L=1024, P=4, HID=1024, NH=16, HS=64)
with RoPE, distributed data-parallel over the 8 (b, p) shards across 8
TRN2 NeuronCores. Self-contained: kernel(**inputs) -> np.ndarray."""

import numpy as np
import ml_dtypes

import concourse.bacc as bacc
import concourse.mybir as mybir
import concourse.tile as tile
from concourse.bass_utils import run_bass_kernel_spmd

B, L, P, HID = 2, 1024, 4, 1024
NH, HS = 16, 64
NCORES = 8
KT = 8          # contraction tiles of 128 over HID
DT = 8          # d-tiles of 128 (2 heads each)
NTT = 2         # token tiles of 512
BF16 = mybir.dt.bfloat16
F32 = mybir.dt.float32
AF = mybir.ActivationFunctionType
ALU = mybir.AluOpType
NPBF16 = ml_dtypes.bfloat16

def build_nc(debug=False):
    nc = bacc.Bacc()
    xT = nc.declare_dram_parameter("xT", [HID, L], BF16, isOutput=False)
    wqT = nc.declare_dram_parameter("wqT", [HID, HID], BF16, isOutput=False)
    wkT = nc.declare_dram_parameter("wkT", [HID, HID], BF16, isOutput=False)
    wvT = nc.declare_dram_parameter("wvT", [HID, HID], BF16, isOutput=False)
    owT = nc.declare_dram_parameter("owT", [HID, HID], BF16, isOutput=False)
    cosT2 = nc.declare_dram_parameter("cosT2", [128, L], BF16, isOutput=False)
    sinT2 = nc.declare_dram_parameter("sinT2", [128, L], BF16, isOutput=False)
    rmat = nc.declare_dram_parameter("rmat", [128, 128], BF16, isOutput=False)
    atri = nc.declare_dram_parameter("atri", [128, 128], BF16, isOutput=False)
    ident = nc.declare_dram_parameter("ident", [128, 128], BF16, isOutput=False)
    niota2 = nc.declare_dram_parameter("niota2", [128, 2, 128], F32, isOutput=False)
    mlen = nc.declare_dram_parameter("mlen", [128, 1], F32, isOutput=False)
    outT = nc.declare_dram_parameter("outT", [HID, L], BF16, isOutput=True)
    dbg = {}
    if debug:
        dbg["qrope"] = nc.declare_dram_parameter("dbg_qrope", [128, DT, L], BF16, isOutput=True)
        dbg["krope"] = nc.declare_dram_parameter("dbg_krope", [128, DT, L], BF16, isOutput=True)
        dbg["v"] = nc.declare_dram_parameter("dbg_v", [128, 8, NH, 65], BF16, isOutput=True)
        dbg["o"] = nc.declare_dram_parameter("dbg_o", [128, DT, L], BF16, isOutput=True)
        dbg["sc"] = nc.declare_dram_parameter("dbg_sc", [128, 2, 512], F32, isOutput=True)
        dbg["es"] = nc.declare_dram_parameter("dbg_es", [128, 2, 512], BF16, isOutput=True)
        dbg["zb"] = nc.declare_dram_parameter("dbg_zb", [64, 512], F32, isOutput=True)
        dbg["rz"] = nc.declare_dram_parameter("dbg_rz", [64, 512], F32, isOutput=True)
        dbg["otraw"] = nc.declare_dram_parameter("dbg_otraw", [65, 512], F32, isOutput=True)

    with tile.TileContext(nc) as tc:
        with tc.tile_pool(name="consts", bufs=1) as cpool:
            xs = cpool.tile([128, KT, L], BF16, name="xs")
            nc.sync.dma_start(out=xs[:], in_=xT.rearrange("(k p) t -> p k t", p=128))
            wqs = cpool.tile([128, KT, HID], BF16, name="wqs")
            nc.sync.dma_start(out=wqs[:], in_=wqT.rearrange("(k p) d -> p k d", p=128))
            wks = cpool.tile([128, KT, HID], BF16, name="wks")
            nc.sync.dma_start(out=wks[:], in_=wkT.rearrange("(k p) d -> p k d", p=128))
            wvs = cpool.tile([128, KT, HID], BF16, name="wvs")
            nc.sync.dma_start(out=wvs[:], in_=wvT.rearrange("(k p) d -> p k d", p=128))
            ows = cpool.tile([128, KT, HID], BF16, name="ows")
            nc.sync.dma_start(out=ows[:], in_=owT.rearrange("(k p) h -> p k h", p=128))
            coss = cpool.tile([128, L], BF16, name="coss")
            nc.sync.dma_start(out=coss[:], in_=cosT2[:])
            sins = cpool.tile([128, L], BF16, name="sins")
            nc.sync.dma_start(out=sins[:], in_=sinT2[:])
            rms = cpool.tile([128, 128], BF16, name="rms")
            nc.sync.dma_start(out=rms[:], in_=rmat[:])
            ats = cpool.tile([128, 128], BF16, name="ats")
            nc.sync.dma_start(out=ats[:], in_=atri[:])
            ids = cpool.tile([128, 128], BF16, name="ids")
            nc.sync.dma_start(out=ids[:], in_=ident[:])
            nio = cpool.tile([128, 2, 128], F32, name="nio")
            nc.sync.dma_start(out=nio[:], in_=niota2[:])
            mls = cpool.tile([128, 1], F32, name="mls")
            nc.sync.dma_start(out=mls[:], in_=mlen[:])

            with tc.tile_pool(name="persist", bufs=1) as ppool:
                qrope = ppool.tile([128, DT, L], BF16, name="qrope")
                krope = ppool.tile([128, DT, L], BF16, name="krope")
                # v in natural layout [tok%128, tok//128, head, hs+ones]
                vs = ppool.tile([128, 8, NH, 128], BF16, name="vs")
                # normalized attention output, transposed: [d%128, head-pair, tok]
                os2 = ppool.tile([128, DT, L], BF16, name="os2")

                nc.gpsimd.memset(vs[:, :, :, 64:128], 0.0)
                nc.gpsimd.memset(vs[:, :, :, 64:65], 1.0)
                ones64 = ppool.tile([65, 64], BF16, name="ones64")
                nc.gpsimd.memset(ones64[:], 1.0)

                # ---------------- Phase 1: projections + rope -------------
                with (
                    tc.tile_pool(name="psum", bufs=1, space="PSUM") as pp,
                    tc.tile_pool(name="work", bufs=1) as wp1,
                ):
                    wp2 = wp1
                    pp1 = pp2 = pp3 = pp
                    # v = x @ wv^T in natural layout
                    for bt in range(8):          # token tile of 128
                        for dh in range(NTT):    # d half of 512 (8 heads)
                            vp = pp1.tile([128, 512], F32, name="vp", tag="A", bufs=2)
                            for k in range(KT):
                                nc.tensor.matmul(
                                    vp[:],
                                    xs[:, k, bt * 128:(bt + 1) * 128],
                                    wvs[:, k, dh * 512:(dh + 1) * 512],
                                    start=(k == 0), stop=(k == KT - 1),
                                )
                            nc.scalar.copy(
                                out=vs[:, bt, dh * 8:(dh + 1) * 8, 0:64],
                                in_=vp[:].rearrange("p (h e) -> p h e", e=64),
                            )
                    # per head-pair group: qk proj + rope, then attention.
                    # The Z-bcast/recip/norm tail of group g is deferred until
                    # after group g+1's projection matmuls are queued, so the
                    # in-order PE never waits on it.
                    def emit_norm(items):
                        for (g_, c_, h_, stge_) in items:
                            zbp = pp2.tile([64, 512], F32, name="zbp",
                                           tag="ot0" if h_ == 0 else "ot1", bufs=1)
                            nc.tensor.matmul(zbp[:], ones64[64:65, :], stge_[64:65, :],
                                             start=True, stop=True)
                            rz = wp2.tile([64, 512], F32, name="rz", tag=f"rz{h_}", bufs=2)
                            nc.vector.reciprocal_approx_fast(out=rz[:], in_=zbp[:])
                            if debug and g_ == 0 and c_ == 0 and h_ == 0:
                                nc.sync.dma_start(out=dbg["rz"][:], in_=rz[:])
                            if h_ == 0:
                                nc.vector.tensor_mul(
                                    os2[0:64, g_, c_ * 512:(c_ + 1) * 512],
                                    stge_[0:64, :],
                                    rz[:],
                                )
                            else:
                                stg = wp2.tile([64, 512], BF16, name="stg", tag="stg", bufs=2)
                                nc.vector.tensor_mul(stg[:], stge_[0:64, :], rz[:])
                                nc.sync.dma_start(
                                    out=os2[64:128, g_, c_ * 512:(c_ + 1) * 512],
                                    in_=stg[:],
                                )

                    pending = []
                    for kd in range(DT):
                        for t in range(NTT):
                            for (ws, dest) in ((wqs, qrope), (wks, krope)):
                                qp = pp1.tile([128, 512], F32, name="qp", tag="qp", bufs=1)
                                for k in range(KT):
                                    nc.tensor.matmul(
                                        qp[:],
                                        ws[:, k, kd * 128:(kd + 1) * 128],
                                        xs[:, k, t * 512:(t + 1) * 512],
                                        start=(k == 0), stop=(k == KT - 1),
                                    )
                                qraw = wp1.tile([128, 512], BF16, name="qraw", tag="qraw", bufs=3)
                                nc.vector.tensor_copy(qraw[:], qp[:])
                                rp = pp1.tile([128, 512], F32, name="rp", tag="rp", bufs=1)
                                nc.tensor.matmul(rp[:], rms[:], qraw[:], start=True, stop=True)
                                m1 = wp1.tile([128, 512], BF16, name="m1", tag="m1", bufs=3)
                                nc.gpsimd.tensor_mul(m1[:], qraw[:], coss[:, t * 512:(t + 1) * 512])
                                m2 = wp1.tile([128, 512], BF16, name="m2", tag="m2", bufs=3)
                                nc.vector.tensor_mul(m2[:], rp[:], sins[:, t * 512:(t + 1) * 512])
                                nc.gpsimd.tensor_add(
                                    dest[:, kd, t * 512:(t + 1) * 512], m1[:], m2[:]
                                )

                        # deferred norm tail of the previous group
                        emit_norm(pending)
                        pending = []

                        # ---- attention for this head pair ----
                        g = kd
                        for c in range(NTT):     # i tile of 512
                            ot = [
                                pp2.tile([128, 512], F32, name=f"ot{h}_{g}_{c}",
                                         tag="ot0" if h == 0 else "ot1", bufs=1)
                                for h in range(2)
                            ]
                            nb = 4 * c + 4

                            def emit_omm(b_, es_, off_):
                                for h in range(2):
                                    nc.tensor.matmul(
                                        ot[h][0:128, off_:512],
                                        vs[:, b_, 2 * g + h, :],
                                        es_[:, h, off_:],
                                        start=(b_ == 0), stop=(b_ == nb - 1),
                                    )

                            prev_omm = None
                            for b in range(nb):  # j block of 128
                                scp = pp2.tile([128, 2, 512], F32, name="scp", tag="A", bufs=2)
                                partial = b >= 4 * c
                                off = max(0, 128 * b - 512 * c)
                                for h in range(2):
                                    nc.tensor.matmul(
                                        scp[:, h, off:],
                                        krope[64 * h:64 * h + 64, g, b * 128:(b + 1) * 128],
                                        qrope[64 * h:64 * h + 64, g, c * 512 + off:(c + 1) * 512],
                                        start=True, stop=not partial,
                                        tile_position=(64 * h, 0),
                                    )
                                if partial:
                                    for h in range(2):
                                        nc.tensor.matmul(
                                            scp[:, h, off:off + 128],
                                            ats[:], ids[:],
                                            start=False, stop=True,
                                        )
                                es = wp2.tile([128, 2, 512], BF16, name="es", tag="es", bufs=4)
                                nc.scalar.activation(
                                    out=es[:, :, off:], in_=scp[:, :, off:],
                                    func=AF.Exp, scale=0.125,
                                )
                                if debug and g == 0 and c == 0 and b == 0:
                                    dsc = ppool.tile([128, 2, 512], F32, name="dsc")
                                    nc.scalar.copy(out=dsc[:], in_=scp[:])
                                    nc.sync.dma_start(out=dbg["sc"][:], in_=dsc[:])
                                    nc.sync.dma_start(out=dbg["es"][:], in_=es[:])
                                if prev_omm is not None:
                                    emit_omm(*prev_omm)
                                prev_omm = (b, es, off)
                            emit_omm(*prev_omm)
                            # drain ot -> SBUF right away (releases the PSUM bank)
                            for h in range(2):
                                stge = wp2.tile([65, 512], BF16, name="stge", tag=f"stge{h}", bufs=2)
                                nc.vector.tensor_copy(stge[:], ot[h][0:65, :])
                                if debug and g == 0 and c == 0 and h == 0:
                                    dot = ppool.tile([65, 512], F32, name="dot")
                                    nc.scalar.copy(out=dot[:], in_=ot[0][0:65, :])
                                    nc.sync.dma_start(out=dbg["otraw"][:], in_=dot[:])
                                pending.append((g, c, h, stge))
                    emit_norm(pending)

                    # ------------- Phase 3: output projection -------------
                    wp3 = wp1
                    outr = outT.rearrange("(k p) t -> p k t", p=128)
                    for kh in range(DT):
                        for t in range(NTT):
                            fp = pp3.tile([128, 512], F32, name="fp", tag="A", bufs=2)
                            for k in range(KT):
                                nc.tensor.matmul(
                                    fp[:],
                                    ows[:, k, kh * 128:(kh + 1) * 128],
                                    os2[:, k, t * 512:(t + 1) * 512],
                                    start=(k == 0), stop=(k == KT - 1),
                                )
                            fo = wp3.tile([128, 512], BF16, name="fo", tag="fo", bufs=3)
                            nc.vector.tensor_copy(fo[:], fp[:])
                            nc.sync.dma_start(
                                out=outr[:, kh, t * 512:(t + 1) * 512], in_=fo[:]
                            )

                if debug:
                    nc.sync.dma_start(out=dbg["qrope"][:], in_=qrope[:])
                    nc.sync.dma_start(out=dbg["krope"][:], in_=krope[:])
                    nc.sync.dma_start(out=dbg["v"][:], in_=vs[:])
                    nc.sync.dma_start(out=dbg["o"][:], in_=os2[:])
    nc.finalize()
    return nc


def _host_consts():
    rmat = np.zeros((128, 128), np.float32)
    for m in range(128):
        if (m % 64) < 32:
            rmat[m + 32, m] = -1.0
        else:
            rmat[m - 32, m] = 1.0
    atri = -240.0 * np.triu(np.ones((128, 128), np.float32), 1)
    ident = np.eye(128, dtype=np.float32)
    niota2 = np.broadcast_to(-np.arange(128, dtype=np.float32), (128, 2, 128)).copy()
    mlen = (1.0 - np.arange(128, dtype=np.float32)).reshape(128, 1)
    return (rmat.astype(NPBF16), atri.astype(NPBF16), ident.astype(NPBF16),
            niota2, mlen)


def kernel(x, qw, kw, vw, ow, cos, sin, debug=False):
    x = np.asarray(x, np.float32)
    qw = np.asarray(qw, np.float32)
    kw = np.asarray(kw, np.float32)
    vw = np.asarray(vw, np.float32)
    ow = np.asarray(ow, np.float32)
    cos = np.asarray(cos, np.float32)
    sin = np.asarray(sin, np.float32)

    wqT = np.ascontiguousarray(qw.T).astype(NPBF16)
    wkT = np.ascontiguousarray(kw.T).astype(NPBF16)
    wvT = np.ascontiguousarray(vw.T).astype(NPBF16)
    owT = np.ascontiguousarray(ow.T).astype(NPBF16)
    cosT2 = np.concatenate([cos.T, cos.T], 0).astype(NPBF16)
    sinT2 = np.concatenate([sin.T, sin.T], 0).astype(NPBF16)
    rmat, atri, ident, niota2, mlen = _host_consts()

    shared = {
        "wqT": wqT, "wkT": wkT, "wvT": wvT, "owT": owT,
        "cosT2": np.ascontiguousarray(cosT2), "sinT2": np.ascontiguousarray(sinT2),
        "rmat": rmat, "atri": atri, "ident": ident,
        "niota2": niota2, "mlen": mlen,
    }
    in_maps = []
    for c in range(NCORES):
        b, p = divmod(c, P)
        xTc = np.ascontiguousarray(x[b, :, p, :].T).astype(NPBF16)
        m = dict(shared)
        m["xT"] = xTc
        in_maps.append(m)

    nc = build_nc(debug=debug)
    res = run_bass_kernel_spmd(nc, in_maps, list(range(NCORES)))

    y = np.empty((B, L, P, HID), np.float32)
    for c in range(NCORES):
        b, p = divmod(c, P)
        y[b, :, p, :] = np.asarray(res.results[c]["outT"], np.float32).T
    if debug:
        return y, res
    return y


# revision 22
# speedup vs baseline: 1.0554x; 1.0244x over previous
"""Causal multi-head attention (B=2, L=1024, P=4, HID=1024, NH=16, HS=64)
with RoPE, distributed data-parallel over the 8 (b, p) shards across 8
TRN2 NeuronCores. Self-contained: kernel(**inputs) -> np.ndarray."""

import numpy as np
import ml_dtypes

import concourse.bacc as bacc
import concourse.mybir as mybir
import concourse.tile as tile
from concourse.bass_utils import run_bass_kernel_spmd

B, L, P, HID = 2, 1024, 4, 1024
NH, HS = 16, 64
NCORES = 8
KT = 8          # contraction tiles of 128 over HID
DT = 8          # d-tiles of 128 (2 heads each)
NTT = 2         # token tiles of 512
BF16 = mybir.dt.bfloat16
F32 = mybir.dt.float32
AF = mybir.ActivationFunctionType
ALU = mybir.AluOpType
NPBF16 = ml_dtypes.bfloat16

def build_nc(debug=False):
    nc = bacc.Bacc()
    xT = nc.declare_dram_parameter("xT", [HID, L], BF16, isOutput=False)
    wqT = nc.declare_dram_parameter("wqT", [HID, HID], BF16, isOutput=False)
    wkT = nc.declare_dram_parameter("wkT", [HID, HID], BF16, isOutput=False)
    wvT = nc.declare_dram_parameter("wvT", [HID, HID], BF16, isOutput=False)
    owT = nc.declare_dram_parameter("owT", [HID, HID], BF16, isOutput=False)
    cosT2 = nc.declare_dram_parameter("cosT2", [128, L], BF16, isOutput=False)
    sinT2 = nc.declare_dram_parameter("sinT2", [128, L], BF16, isOutput=False)
    rmat = nc.declare_dram_parameter("rmat", [128, 128], BF16, isOutput=False)
    atri = nc.declare_dram_parameter("atri", [128, 128], BF16, isOutput=False)
    ident = nc.declare_dram_parameter("ident", [128, 128], BF16, isOutput=False)
    niota2 = nc.declare_dram_parameter("niota2", [128, 2, 128], F32, isOutput=False)
    mlen = nc.declare_dram_parameter("mlen", [128, 1], F32, isOutput=False)
    outT = nc.declare_dram_parameter("outT", [HID, L], BF16, isOutput=True)
    dbg = {}
    if debug:
        dbg["qrope"] = nc.declare_dram_parameter("dbg_qrope", [128, DT, L], BF16, isOutput=True)
        dbg["krope"] = nc.declare_dram_parameter("dbg_krope", [128, DT, L], BF16, isOutput=True)
        dbg["v"] = nc.declare_dram_parameter("dbg_v", [128, 8, NH, 65], BF16, isOutput=True)
        dbg["o"] = nc.declare_dram_parameter("dbg_o", [128, DT, L], BF16, isOutput=True)
        dbg["sc"] = nc.declare_dram_parameter("dbg_sc", [128, 2, 512], F32, isOutput=True)
        dbg["es"] = nc.declare_dram_parameter("dbg_es", [128, 2, 512], BF16, isOutput=True)
        dbg["zb"] = nc.declare_dram_parameter("dbg_zb", [64, 512], F32, isOutput=True)
        dbg["rz"] = nc.declare_dram_parameter("dbg_rz", [64, 512], F32, isOutput=True)
        dbg["otraw"] = nc.declare_dram_parameter("dbg_otraw", [65, 512], F32, isOutput=True)

    with tile.TileContext(nc) as tc:
        with tc.tile_pool(name="consts", bufs=1) as cpool:
            xs = cpool.tile([128, KT, L], BF16, name="xs")
            wqs = cpool.tile([128, KT, HID], BF16, name="wqs")
            wks = cpool.tile([128, KT, HID], BF16, name="wks")
            wvs = cpool.tile([128, KT, HID], BF16, name="wvs")
            ows = cpool.tile([128, KT, HID], BF16, name="ows")
            xr = xT.rearrange("(k p) t -> p k t", p=128)
            wqr = wqT.rearrange("(k p) d -> p k d", p=128)
            wkr = wkT.rearrange("(k p) d -> p k d", p=128)
            wvr = wvT.rearrange("(k p) d -> p k d", p=128)
            owr = owT.rearrange("(k p) h -> p k h", p=128)
            # per-k-tile DMAs so the first matmuls start as soon as possible;
            # xs/wvs first (v-projection runs first), then wq/wk, ow last.
            for k in range(KT):
                nc.sync.dma_start(out=xs[:, k, :], in_=xr[:, k, :])
                nc.sync.dma_start(out=wvs[:, k, :], in_=wvr[:, k, :])
            for k in range(KT):
                nc.sync.dma_start(out=wqs[:, k, :], in_=wqr[:, k, :])
                nc.sync.dma_start(out=wks[:, k, :], in_=wkr[:, k, :])
            nc.sync.dma_start(out=ows[:], in_=owr)
            coss = cpool.tile([128, L], BF16, name="coss")
            nc.sync.dma_start(out=coss[:], in_=cosT2[:])
            sins = cpool.tile([128, L], BF16, name="sins")
            nc.sync.dma_start(out=sins[:], in_=sinT2[:])
            rms = cpool.tile([128, 128], BF16, name="rms")
            nc.sync.dma_start(out=rms[:], in_=rmat[:])
            ats = cpool.tile([128, 128], BF16, name="ats")
            nc.sync.dma_start(out=ats[:], in_=atri[:])
            ids = cpool.tile([128, 128], BF16, name="ids")
            nc.sync.dma_start(out=ids[:], in_=ident[:])
            nio = cpool.tile([128, 2, 128], F32, name="nio")
            nc.sync.dma_start(out=nio[:], in_=niota2[:])
            mls = cpool.tile([128, 1], F32, name="mls")
            nc.sync.dma_start(out=mls[:], in_=mlen[:])

            with tc.tile_pool(name="persist", bufs=1) as ppool:
                qrope = ppool.tile([128, DT, L], BF16, name="qrope")
                krope = ppool.tile([128, DT, L], BF16, name="krope")
                # v in natural layout [tok%128, tok//128, head, hs+ones]
                vs = ppool.tile([128, 8, NH, 128], BF16, name="vs")
                # normalized attention output, transposed: [d%128, head-pair, tok]
                os2 = ppool.tile([128, DT, L], BF16, name="os2")

                nc.gpsimd.memset(vs[:, :, :, 64:128], 0.0)
                nc.gpsimd.memset(vs[:, :, :, 64:65], 1.0)
                ones64 = ppool.tile([65, 64], BF16, name="ones64")
                nc.gpsimd.memset(ones64[:], 1.0)

                # ---------------- Phase 1: projections + rope -------------
                with (
                    tc.tile_pool(name="psum", bufs=1, space="PSUM") as pp,
                    tc.tile_pool(name="work", bufs=1) as wp1,
                ):
                    wp2 = wp1
                    pp1 = pp2 = pp3 = pp
                    # v = x @ wv^T in natural layout
                    for bt in range(8):          # token tile of 128
                        for dh in range(NTT):    # d half of 512 (8 heads)
                            vp = pp1.tile([128, 512], F32, name="vp", tag="A", bufs=2)
                            for k in range(KT):
                                nc.tensor.matmul(
                                    vp[:],
                                    xs[:, k, bt * 128:(bt + 1) * 128],
                                    wvs[:, k, dh * 512:(dh + 1) * 512],
                                    start=(k == 0), stop=(k == KT - 1),
                                )
                            nc.scalar.copy(
                                out=vs[:, bt, dh * 8:(dh + 1) * 8, 0:64],
                                in_=vp[:].rearrange("p (h e) -> p h e", e=64),
                            )
                    # per head-pair group: qk proj + rope, then attention.
                    # The Z-bcast/recip/norm tail of group g is deferred until
                    # after group g+1's projection matmuls are queued, so the
                    # in-order PE never waits on it.
                    def emit_norm(items):
                        for (g_, c_, h_, stge_) in items:
                            zbp = pp2.tile([64, 512], F32, name="zbp",
                                           tag="ot0" if h_ == 0 else "ot1", bufs=1)
                            nc.tensor.matmul(zbp[:], ones64[64:65, :], stge_[64:65, :],
                                             start=True, stop=True)
                            rz = wp2.tile([64, 512], F32, name="rz", tag=f"rz{h_}", bufs=2)
                            nc.vector.reciprocal_approx_fast(out=rz[:], in_=zbp[:])
                            if debug and g_ == 0 and c_ == 0 and h_ == 0:
                                nc.sync.dma_start(out=dbg["rz"][:], in_=rz[:])
                            if h_ == 0:
                                nc.vector.tensor_mul(
                                    os2[0:64, g_, c_ * 512:(c_ + 1) * 512],
                                    stge_[0:64, :],
                                    rz[:],
                                )
                            else:
                                stg = wp2.tile([64, 512], BF16, name="stg", tag="stg", bufs=2)
                                nc.vector.tensor_mul(stg[:], stge_[0:64, :], rz[:])
                                nc.sync.dma_start(
                                    out=os2[64:128, g_, c_ * 512:(c_ + 1) * 512],
                                    in_=stg[:],
                                )

                    pending = []
                    for kd in range(DT):
                        for t in range(NTT):
                            for (ws, dest) in ((wqs, qrope), (wks, krope)):
                                qp = pp1.tile([128, 512], F32, name="qp", tag="qp", bufs=1)
                                for k in range(KT):
                                    nc.tensor.matmul(
                                        qp[:],
                                        ws[:, k, kd * 128:(kd + 1) * 128],
                                        xs[:, k, t * 512:(t + 1) * 512],
                                        start=(k == 0), stop=(k == KT - 1),
                                    )
                                qraw = wp1.tile([128, 512], BF16, name="qraw", tag="qraw", bufs=3)
                                nc.vector.tensor_copy(qraw[:], qp[:])
                                rp = pp1.tile([128, 512], F32, name="rp", tag="rp", bufs=1)
                                nc.tensor.matmul(rp[:], rms[:], qraw[:], start=True, stop=True)
                                m1 = wp1.tile([128, 512], BF16, name="m1", tag="m1", bufs=3)
                                nc.gpsimd.tensor_mul(m1[:], qraw[:], coss[:, t * 512:(t + 1) * 512])
                                m2 = wp1.tile([128, 512], BF16, name="m2", tag="m2", bufs=3)
                                nc.vector.tensor_mul(m2[:], rp[:], sins[:, t * 512:(t + 1) * 512])
                                nc.gpsimd.tensor_add(
                                    dest[:, kd, t * 512:(t + 1) * 512], m1[:], m2[:]
                                )

                        # deferred norm tail of the previous group
                        emit_norm(pending)
                        pending = []

                        # ---- attention for this head pair ----
                        g = kd
                        for c in range(NTT):     # i tile of 512
                            ot = [
                                pp2.tile([128, 512], F32, name=f"ot{h}_{g}_{c}",
                                         tag="ot0" if h == 0 else "ot1", bufs=1)
                                for h in range(2)
                            ]
                            nb = 4 * c + 4

                            def emit_omm(b_, es_, off_):
                                for h in range(2):
                                    nc.tensor.matmul(
                                        ot[h][0:128, off_:512],
                                        vs[:, b_, 2 * g + h, :],
                                        es_[:, h, off_:],
                                        start=(b_ == 0), stop=(b_ == nb - 1),
                                    )

                            prev_omm = None
                            for b in range(nb):  # j block of 128
                                scp = pp2.tile([128, 2, 512], F32, name="scp", tag="A", bufs=2)
                                partial = b >= 4 * c
                                off = max(0, 128 * b - 512 * c)
                                for h in range(2):
                                    nc.tensor.matmul(
                                        scp[:, h, off:],
                                        krope[64 * h:64 * h + 64, g, b * 128:(b + 1) * 128],
                                        qrope[64 * h:64 * h + 64, g, c * 512 + off:(c + 1) * 512],
                                        start=True, stop=not partial,
                                        tile_position=(64 * h, 0),
                                    )
                                if partial:
                                    for h in range(2):
                                        nc.tensor.matmul(
                                            scp[:, h, off:off + 128],
                                            ats[:], ids[:],
                                            start=False, stop=True,
                                        )
                                es = wp2.tile([128, 2, 512], BF16, name="es", tag="es", bufs=4)
                                nc.scalar.activation(
                                    out=es[:, :, off:], in_=scp[:, :, off:],
                                    func=AF.Exp, scale=0.125,
                                )
                                if debug and g == 0 and c == 0 and b == 0:
                                    dsc = ppool.tile([128, 2, 512], F32, name="dsc")
                                    nc.scalar.copy(out=dsc[:], in_=scp[:])
                                    nc.sync.dma_start(out=dbg["sc"][:], in_=dsc[:])
                                    nc.sync.dma_start(out=dbg["es"][:], in_=es[:])
                                if prev_omm is not None:
                                    emit_omm(*prev_omm)
                                prev_omm = (b, es, off)
                            emit_omm(*prev_omm)
                            # drain ot -> SBUF right away (releases the PSUM bank)
                            for h in range(2):
                                stge = wp2.tile([65, 512], BF16, name="stge", tag=f"stge{h}", bufs=2)
                                nc.vector.tensor_copy(stge[:], ot[h][0:65, :])
                                if debug and g == 0 and c == 0 and h == 0:
                                    dot = ppool.tile([65, 512], F32, name="dot")
                                    nc.scalar.copy(out=dot[:], in_=ot[0][0:65, :])
                                    nc.sync.dma_start(out=dbg["otraw"][:], in_=dot[:])
                                pending.append((g, c, h, stge))
                    emit_norm(pending)

                    # ------------- Phase 3: output projection -------------
                    wp3 = wp1
                    outr = outT.rearrange("(k p) t -> p k t", p=128)
                    for kh in range(DT):
                        for t in range(NTT):
                            fp = pp3.tile([128, 512], F32, name="fp", tag="A", bufs=2)
                            for k in range(KT):
                                nc.tensor.matmul(
                                    fp[:],
                                    ows[:, k, kh * 128:(kh + 1) * 128],
                                    os2[:, k, t * 512:(t + 1) * 512],
                                    start=(k == 0), stop=(k == KT - 1),
                                )
                            fo = wp3.tile([128, 512], BF16, name="fo", tag="fo", bufs=3)
                            nc.vector.tensor_copy(fo[:], fp[:])
                            nc.sync.dma_start(
                                out=outr[:, kh, t * 512:(t + 1) * 512], in_=fo[:]
                            )

                if debug:
                    nc.sync.dma_start(out=dbg["qrope"][:], in_=qrope[:])
                    nc.sync.dma_start(out=dbg["krope"][:], in_=krope[:])
                    nc.sync.dma_start(out=dbg["v"][:], in_=vs[:])
                    nc.sync.dma_start(out=dbg["o"][:], in_=os2[:])
    nc.finalize()
    return nc


def _host_consts():
    rmat = np.zeros((128, 128), np.float32)
    for m in range(128):
        if (m % 64) < 32:
            rmat[m + 32, m] = -1.0
        else:
            rmat[m - 32, m] = 1.0
    atri = -240.0 * np.triu(np.ones((128, 128), np.float32), 1)
    ident = np.eye(128, dtype=np.float32)
    niota2 = np.broadcast_to(-np.arange(128, dtype=np.float32), (128, 2, 128)).copy()
    mlen = (1.0 - np.arange(128, dtype=np.float32)).reshape(128, 1)
    return (rmat.astype(NPBF16), atri.astype(NPBF16), ident.astype(NPBF16),
            niota2, mlen)


def kernel(x, qw, kw, vw, ow, cos, sin, debug=False):
    x = np.asarray(x, np.float32)
    qw = np.asarray(qw, np.float32)
    kw = np.asarray(kw, np.float32)
    vw = np.asarray(vw, np.float32)
    ow = np.asarray(ow, np.float32)
    cos = np.asarray(cos, np.float32)
    sin = np.asarray(sin, np.float32)

    wqT = np.ascontiguousarray(qw.T).astype(NPBF16)
    wkT = np.ascontiguousarray(kw.T).astype(NPBF16)
    wvT = np.ascontiguousarray(vw.T).astype(NPBF16)
    owT = np.ascontiguousarray(ow.T).astype(NPBF16)
    cosT2 = np.concatenate([cos.T, cos.T], 0).astype(NPBF16)
    sinT2 = np.concatenate([sin.T, sin.T], 0).astype(NPBF16)
    rmat, atri, ident, niota2, mlen = _host_consts()

    shared = {
        "wqT": wqT, "wkT": wkT, "wvT": wvT, "owT": owT,
        "cosT2": np.ascontiguousarray(cosT2), "sinT2": np.ascontiguousarray(sinT2),
        "rmat": rmat, "atri": atri, "ident": ident,
        "niota2": niota2, "mlen": mlen,
    }
    in_maps = []
    for c in range(NCORES):
        b, p = divmod(c, P)
        xTc = np.ascontiguousarray(x[b, :, p, :].T).astype(NPBF16)
        m = dict(shared)
        m["xT"] = xTc
        in_maps.append(m)

    nc = build_nc(debug=debug)
    res = run_bass_kernel_spmd(nc, in_maps, list(range(NCORES)))

    y = np.empty((B, L, P, HID), np.float32)
    for c in range(NCORES):
        b, p = divmod(c, P)
        y[b, :, p, :] = np.asarray(res.results[c]["outT"], np.float32).T
    if debug:
        return y, res
    return y


# revision 23
# speedup vs baseline: 1.0576x; 1.0021x over previous
"""Causal multi-head attention (B=2, L=1024, P=4, HID=1024, NH=16, HS=64)
with RoPE, distributed data-parallel over the 8 (b, p) shards across 8
TRN2 NeuronCores. Self-contained: kernel(**inputs) -> np.ndarray."""

import numpy as np
import ml_dtypes

import concourse.bacc as bacc
import concourse.mybir as mybir
import concourse.tile as tile
from concourse.bass_utils import run_bass_kernel_spmd

B, L, P, HID = 2, 1024, 4, 1024
NH, HS = 16, 64
NCORES = 8
KT = 8          # contraction tiles of 128 over HID
DT = 8          # d-tiles of 128 (2 heads each)
NTT = 2         # token tiles of 512
BF16 = mybir.dt.bfloat16
F32 = mybir.dt.float32
AF = mybir.ActivationFunctionType
ALU = mybir.AluOpType
NPBF16 = ml_dtypes.bfloat16

def build_nc(debug=False):
    nc = bacc.Bacc()
    xT = nc.declare_dram_parameter("xT", [HID, L], BF16, isOutput=False)
    wqT = nc.declare_dram_parameter("wqT", [HID, HID], BF16, isOutput=False)
    wkT = nc.declare_dram_parameter("wkT", [HID, HID], BF16, isOutput=False)
    wvT = nc.declare_dram_parameter("wvT", [HID, HID], BF16, isOutput=False)
    owT = nc.declare_dram_parameter("owT", [HID, HID], BF16, isOutput=False)
    cosT2 = nc.declare_dram_parameter("cosT2", [128, L], BF16, isOutput=False)
    sinT2 = nc.declare_dram_parameter("sinT2", [128, L], BF16, isOutput=False)
    rmat = nc.declare_dram_parameter("rmat", [128, 128], BF16, isOutput=False)
    atri = nc.declare_dram_parameter("atri", [128, 128], BF16, isOutput=False)
    ident = nc.declare_dram_parameter("ident", [128, 128], BF16, isOutput=False)
    outT = nc.declare_dram_parameter("outT", [HID, L], BF16, isOutput=True)
    dbg = {}
    if debug:
        dbg["qrope"] = nc.declare_dram_parameter("dbg_qrope", [128, DT, L], BF16, isOutput=True)
        dbg["krope"] = nc.declare_dram_parameter("dbg_krope", [128, DT, L], BF16, isOutput=True)
        dbg["v"] = nc.declare_dram_parameter("dbg_v", [128, 8, NH, 65], BF16, isOutput=True)
        dbg["o"] = nc.declare_dram_parameter("dbg_o", [128, DT, L], BF16, isOutput=True)
        dbg["sc"] = nc.declare_dram_parameter("dbg_sc", [128, 2, 512], F32, isOutput=True)
        dbg["es"] = nc.declare_dram_parameter("dbg_es", [128, 2, 512], BF16, isOutput=True)
        dbg["zb"] = nc.declare_dram_parameter("dbg_zb", [64, 512], F32, isOutput=True)
        dbg["rz"] = nc.declare_dram_parameter("dbg_rz", [64, 512], F32, isOutput=True)
        dbg["otraw"] = nc.declare_dram_parameter("dbg_otraw", [65, 512], F32, isOutput=True)

    with tile.TileContext(nc) as tc:
        with tc.tile_pool(name="consts", bufs=1) as cpool:
            xs = cpool.tile([128, KT, L], BF16, name="xs")
            wqs = cpool.tile([128, KT, HID], BF16, name="wqs")
            wks = cpool.tile([128, KT, HID], BF16, name="wks")
            wvs = cpool.tile([128, KT, HID], BF16, name="wvs")
            ows = cpool.tile([128, KT, HID], BF16, name="ows")
            xr = xT.rearrange("(k p) t -> p k t", p=128)
            wqr = wqT.rearrange("(k p) d -> p k d", p=128)
            wkr = wkT.rearrange("(k p) d -> p k d", p=128)
            wvr = wvT.rearrange("(k p) d -> p k d", p=128)
            owr = owT.rearrange("(k p) h -> p k h", p=128)
            # per-k-tile DMAs so the first matmuls start as soon as possible;
            # xs/wvs first (v-projection runs first), then wq/wk, ow last.
            for k in range(KT):
                nc.sync.dma_start(out=xs[:, k, :], in_=xr[:, k, :])
                nc.sync.dma_start(out=wvs[:, k, :], in_=wvr[:, k, :])
            for k in range(KT):
                nc.sync.dma_start(out=wqs[:, k, :], in_=wqr[:, k, :])
                nc.sync.dma_start(out=wks[:, k, :], in_=wkr[:, k, :])
            nc.sync.dma_start(out=ows[:], in_=owr)
            coss = cpool.tile([128, L], BF16, name="coss")
            nc.sync.dma_start(out=coss[:], in_=cosT2[:])
            sins = cpool.tile([128, L], BF16, name="sins")
            nc.sync.dma_start(out=sins[:], in_=sinT2[:])
            rms = cpool.tile([128, 128], BF16, name="rms")
            nc.sync.dma_start(out=rms[:], in_=rmat[:])
            ats = cpool.tile([128, 128], BF16, name="ats")
            nc.sync.dma_start(out=ats[:], in_=atri[:])
            ids = cpool.tile([128, 128], BF16, name="ids")
            nc.sync.dma_start(out=ids[:], in_=ident[:])

            with tc.tile_pool(name="persist", bufs=1) as ppool:
                qrope = ppool.tile([128, DT, L], BF16, name="qrope")
                krope = ppool.tile([128, DT, L], BF16, name="krope")
                # v in natural layout [tok%128, tok//128, head, hs+ones]
                vs = ppool.tile([128, 8, NH, 128], BF16, name="vs")
                # normalized attention output, transposed: [d%128, head-pair, tok]
                os2 = ppool.tile([128, DT, L], BF16, name="os2")

                nc.gpsimd.memset(vs[:, :, :, 64:128], 0.0)
                nc.gpsimd.memset(vs[:, :, :, 64:65], 1.0)
                ones64 = ppool.tile([65, 64], BF16, name="ones64")
                nc.gpsimd.memset(ones64[:], 1.0)

                # ---------------- Phase 1: projections + rope -------------
                with (
                    tc.tile_pool(name="psum", bufs=1, space="PSUM") as pp,
                    tc.tile_pool(name="work", bufs=1) as wp1,
                ):
                    wp2 = wp1
                    pp1 = pp2 = pp3 = pp
                    # v = x @ wv^T in natural layout
                    for bt in range(8):          # token tile of 128
                        for dh in range(NTT):    # d half of 512 (8 heads)
                            vp = pp1.tile([128, 512], F32, name="vp", tag="A", bufs=2)
                            for k in range(KT):
                                nc.tensor.matmul(
                                    vp[:],
                                    xs[:, k, bt * 128:(bt + 1) * 128],
                                    wvs[:, k, dh * 512:(dh + 1) * 512],
                                    start=(k == 0), stop=(k == KT - 1),
                                )
                            nc.scalar.copy(
                                out=vs[:, bt, dh * 8:(dh + 1) * 8, 0:64],
                                in_=vp[:].rearrange("p (h e) -> p h e", e=64),
                            )
                    # per head-pair group: qk proj + rope, then attention.
                    # The Z-bcast/recip/norm tail of group g is deferred until
                    # after group g+1's projection matmuls are queued, so the
                    # in-order PE never waits on it.
                    def emit_norm(items):
                        for (g_, c_, h_, stge_) in items:
                            zbp = pp2.tile([64, 512], F32, name="zbp",
                                           tag="ot0" if h_ == 0 else "ot1", bufs=1)
                            nc.tensor.matmul(zbp[:], ones64[64:65, :], stge_[64:65, :],
                                             start=True, stop=True)
                            rz = wp2.tile([64, 512], F32, name="rz", tag=f"rz{h_}", bufs=2)
                            nc.vector.reciprocal_approx_fast(out=rz[:], in_=zbp[:])
                            if debug and g_ == 0 and c_ == 0 and h_ == 0:
                                nc.sync.dma_start(out=dbg["rz"][:], in_=rz[:])
                            if h_ == 0:
                                nc.vector.tensor_mul(
                                    os2[0:64, g_, c_ * 512:(c_ + 1) * 512],
                                    stge_[0:64, :],
                                    rz[:],
                                )
                            else:
                                stg = wp2.tile([64, 512], BF16, name="stg", tag="stg", bufs=2)
                                nc.vector.tensor_mul(stg[:], stge_[0:64, :], rz[:])
                                nc.sync.dma_start(
                                    out=os2[64:128, g_, c_ * 512:(c_ + 1) * 512],
                                    in_=stg[:],
                                )

                    pending = []
                    for kd in range(DT):
                        for t in range(NTT):
                            for (ws, dest) in ((wqs, qrope), (wks, krope)):
                                qp = pp1.tile([128, 512], F32, name="qp", tag="qp", bufs=1)
                                for k in range(KT):
                                    nc.tensor.matmul(
                                        qp[:],
                                        ws[:, k, kd * 128:(kd + 1) * 128],
                                        xs[:, k, t * 512:(t + 1) * 512],
                                        start=(k == 0), stop=(k == KT - 1),
                                    )
                                qraw = wp1.tile([128, 512], BF16, name="qraw", tag="qraw", bufs=3)
                                nc.vector.tensor_copy(qraw[:], qp[:])
                                rp = pp1.tile([128, 512], F32, name="rp", tag="rp", bufs=1)
                                nc.tensor.matmul(rp[:], rms[:], qraw[:], start=True, stop=True)
                                m1 = wp1.tile([128, 512], BF16, name="m1", tag="m1", bufs=3)
                                nc.gpsimd.tensor_mul(m1[:], qraw[:], coss[:, t * 512:(t + 1) * 512])
                                m2 = wp1.tile([128, 512], BF16, name="m2", tag="m2", bufs=3)
                                nc.vector.tensor_mul(m2[:], rp[:], sins[:, t * 512:(t + 1) * 512])
                                nc.gpsimd.tensor_add(
                                    dest[:, kd, t * 512:(t + 1) * 512], m1[:], m2[:]
                                )

                        # deferred norm tail of the previous group
                        emit_norm(pending)
                        pending = []

                        # ---- attention for this head pair ----
                        g = kd
                        for c in range(NTT):     # i tile of 512
                            ot = [
                                pp2.tile([128, 512], F32, name=f"ot{h}_{g}_{c}",
                                         tag="ot0" if h == 0 else "ot1", bufs=1)
                                for h in range(2)
                            ]
                            nb = 4 * c + 4

                            def emit_omm(b_, es_, off_):
                                for h in range(2):
                                    nc.tensor.matmul(
                                        ot[h][0:128, off_:512],
                                        vs[:, b_, 2 * g + h, :],
                                        es_[:, h, off_:],
                                        start=(b_ == 0), stop=(b_ == nb - 1),
                                    )

                            prev_omm = None
                            for b in range(nb):  # j block of 128
                                scp = pp2.tile([128, 2, 512], F32, name="scp", tag="A", bufs=2)
                                partial = b >= 4 * c
                                off = max(0, 128 * b - 512 * c)
                                for h in range(2):
                                    nc.tensor.matmul(
                                        scp[:, h, off:],
                                        krope[64 * h:64 * h + 64, g, b * 128:(b + 1) * 128],
                                        qrope[64 * h:64 * h + 64, g, c * 512 + off:(c + 1) * 512],
                                        start=True, stop=not partial,
                                        tile_position=(64 * h, 0),
                                    )
                                if partial:
                                    for h in range(2):
                                        nc.tensor.matmul(
                                            scp[:, h, off:off + 128],
                                            ats[:], ids[:],
                                            start=False, stop=True,
                                        )
                                es = wp2.tile([128, 2, 512], BF16, name="es", tag="es", bufs=4)
                                nc.scalar.activation(
                                    out=es[:, :, off:], in_=scp[:, :, off:],
                                    func=AF.Exp, scale=0.125,
                                )
                                if debug and g == 0 and c == 0 and b == 0:
                                    dsc = ppool.tile([128, 2, 512], F32, name="dsc")
                                    nc.scalar.copy(out=dsc[:], in_=scp[:])
                                    nc.sync.dma_start(out=dbg["sc"][:], in_=dsc[:])
                                    nc.sync.dma_start(out=dbg["es"][:], in_=es[:])
                                if prev_omm is not None:
                                    emit_omm(*prev_omm)
                                prev_omm = (b, es, off)
                            emit_omm(*prev_omm)
                            # drain ot -> SBUF right away (releases the PSUM bank)
                            for h in range(2):
                                stge = wp2.tile([65, 512], BF16, name="stge", tag=f"stge{h}", bufs=2)
                                nc.vector.tensor_copy(stge[:], ot[h][0:65, :])
                                if debug and g == 0 and c == 0 and h == 0:
                                    dot = ppool.tile([65, 512], F32, name="dot")
                                    nc.scalar.copy(out=dot[:], in_=ot[0][0:65, :])
                                    nc.sync.dma_start(out=dbg["otraw"][:], in_=dot[:])
                                pending.append((g, c, h, stge))
                    emit_norm(pending)

                    # ------------- Phase 3: output projection -------------
                    wp3 = wp1
                    outr = outT.rearrange("(k p) t -> p k t", p=128)
                    for kh in range(DT):
                        for t in range(NTT):
                            fp = pp3.tile([128, 512], F32, name="fp", tag="A", bufs=2)
                            for k in range(KT):
                                nc.tensor.matmul(
                                    fp[:],
                                    ows[:, k, kh * 128:(kh + 1) * 128],
                                    os2[:, k, t * 512:(t + 1) * 512],
                                    start=(k == 0), stop=(k == KT - 1),
                                )
                            fo = wp3.tile([128, 512], BF16, name="fo", tag="fo", bufs=3)
                            nc.vector.tensor_copy(fo[:], fp[:])
                            nc.sync.dma_start(
                                out=outr[:, kh, t * 512:(t + 1) * 512], in_=fo[:]
                            )

                if debug:
                    nc.sync.dma_start(out=dbg["qrope"][:], in_=qrope[:])
                    nc.sync.dma_start(out=dbg["krope"][:], in_=krope[:])
                    nc.sync.dma_start(out=dbg["v"][:], in_=vs[:])
                    nc.sync.dma_start(out=dbg["o"][:], in_=os2[:])
    nc.finalize()
    return nc


def _host_consts():
    rmat = np.zeros((128, 128), np.float32)
    for m in range(128):
        if (m % 64) < 32:
            rmat[m + 32, m] = -1.0
        else:
            rmat[m - 32, m] = 1.0
    atri = -240.0 * np.triu(np.ones((128, 128), np.float32), 1)
    ident = np.eye(128, dtype=np.float32)
    return (rmat.astype(NPBF16), atri.astype(NPBF16), ident.astype(NPBF16))


def kernel(x, qw, kw, vw, ow, cos, sin, debug=False):
    x = np.asarray(x, np.float32)
    qw = np.asarray(qw, np.float32)
    kw = np.asarray(kw, np.float32)
    vw = np.asarray(vw, np.float32)
    ow = np.asarray(ow, np.float32)
    cos = np.asarray(cos, np.float32)
    sin = np.asarray(sin, np.float32)

    wqT = np.ascontiguousarray(qw.T).astype(NPBF16)
    wkT = np.ascontiguousarray(kw.T).astype(NPBF16)
    wvT = np.ascontiguousarray(vw.T).astype(NPBF16)
    owT = np.ascontiguousarray(ow.T).astype(NPBF16)
    cosT2 = np.concatenate([cos.T, cos.T], 0).astype(NPBF16)
    sinT2 = np.concatenate([sin.T, sin.T], 0).astype(NPBF16)
    rmat, atri, ident = _host_consts()

    shared = {
        "wqT": wqT, "wkT": wkT, "wvT": wvT, "owT": owT,
        "cosT2": np.ascontiguousarray(cosT2), "sinT2": np.ascontiguousarray(sinT2),
        "rmat": rmat, "atri": atri, "ident": ident,
    }
    in_maps = []
    for c in range(NCORES):
        b, p = divmod(c, P)
        xTc = np.ascontiguousarray(x[b, :, p, :].T).astype(NPBF16)
        m = dict(shared)
        m["xT"] = xTc
        in_maps.append(m)

    nc = build_nc(debug=debug)
    res = run_bass_kernel_spmd(nc, in_maps, list(range(NCORES)))

    y = np.empty((B, L, P, HID), np.float32)
    for c in range(NCORES):
        b, p = divmod(c, P)
        y[b, :, p, :] = np.asarray(res.results[c]["outT"], np.float32).T
    if debug:
        return y, res
    return y


# revision 24
# speedup vs baseline: 1.0663x; 1.0082x over previous
"""Causal multi-head attention (B=2, L=1024, P=4, HID=1024, NH=16, HS=64)
with RoPE, distributed data-parallel over the 8 (b, p) shards across 8
TRN2 NeuronCores. Self-contained: kernel(**inputs) -> np.ndarray."""

import numpy as np
import ml_dtypes

import concourse.bacc as bacc
import concourse.mybir as mybir
import concourse.tile as tile
from concourse.bass_utils import run_bass_kernel_spmd

B, L, P, HID = 2, 1024, 4, 1024
NH, HS = 16, 64
NCORES = 8
KT = 8          # contraction tiles of 128 over HID
DT = 8          # d-tiles of 128 (2 heads each)
NTT = 2         # token tiles of 512
BF16 = mybir.dt.bfloat16
F32 = mybir.dt.float32
AF = mybir.ActivationFunctionType
ALU = mybir.AluOpType
NPBF16 = ml_dtypes.bfloat16

def build_nc(debug=False):
    nc = bacc.Bacc()
    xT = nc.declare_dram_parameter("xT", [HID, L], BF16, isOutput=False)
    wqT = nc.declare_dram_parameter("wqT", [HID, HID], BF16, isOutput=False)
    wkT = nc.declare_dram_parameter("wkT", [HID, HID], BF16, isOutput=False)
    wvT = nc.declare_dram_parameter("wvT", [HID, HID], BF16, isOutput=False)
    owT = nc.declare_dram_parameter("owT", [HID, HID], BF16, isOutput=False)
    cosT2 = nc.declare_dram_parameter("cosT2", [128, L], BF16, isOutput=False)
    sinT2 = nc.declare_dram_parameter("sinT2", [128, L], BF16, isOutput=False)
    rmat = nc.declare_dram_parameter("rmat", [128, 128], BF16, isOutput=False)
    atri = nc.declare_dram_parameter("atri", [128, 128], BF16, isOutput=False)
    ident = nc.declare_dram_parameter("ident", [128, 128], BF16, isOutput=False)
    outT = nc.declare_dram_parameter("outT", [HID, L], BF16, isOutput=True)
    dbg = {}
    if debug:
        dbg["qrope"] = nc.declare_dram_parameter("dbg_qrope", [128, DT, L], BF16, isOutput=True)
        dbg["krope"] = nc.declare_dram_parameter("dbg_krope", [128, DT, L], BF16, isOutput=True)
        dbg["v"] = nc.declare_dram_parameter("dbg_v", [128, 8, NH, 128], BF16, isOutput=True)
        dbg["o"] = nc.declare_dram_parameter("dbg_o", [128, DT, L], BF16, isOutput=True)
        dbg["sc"] = nc.declare_dram_parameter("dbg_sc", [128, 2, 512], F32, isOutput=True)
        dbg["es"] = nc.declare_dram_parameter("dbg_es", [128, 2, 512], BF16, isOutput=True)
        dbg["zb"] = nc.declare_dram_parameter("dbg_zb", [64, 512], F32, isOutput=True)
        dbg["rz"] = nc.declare_dram_parameter("dbg_rz", [64, 512], F32, isOutput=True)
        dbg["otraw"] = nc.declare_dram_parameter("dbg_otraw", [65, 512], F32, isOutput=True)

    with tile.TileContext(nc) as tc:
        with tc.tile_pool(name="consts", bufs=1) as cpool:
            xs = cpool.tile([128, KT, L], BF16, name="xs")
            wqs = cpool.tile([128, KT, HID], BF16, name="wqs")
            wks = cpool.tile([128, KT, HID], BF16, name="wks")
            wvs = cpool.tile([128, KT, HID], BF16, name="wvs")
            ows = cpool.tile([128, KT, HID], BF16, name="ows")
            xr = xT.rearrange("(k p) t -> p k t", p=128)
            wqr = wqT.rearrange("(k p) d -> p k d", p=128)
            wkr = wkT.rearrange("(k p) d -> p k d", p=128)
            wvr = wvT.rearrange("(k p) d -> p k d", p=128)
            owr = owT.rearrange("(k p) h -> p k h", p=128)
            # per-k-tile DMAs so the first matmuls start as soon as possible;
            # xs/wvs first (v-projection runs first), then wq/wk, ow last.
            for k in range(KT):
                nc.sync.dma_start(out=xs[:, k, :], in_=xr[:, k, :])
                nc.sync.dma_start(out=wvs[:, k, :], in_=wvr[:, k, :])
            for k in range(KT):
                nc.sync.dma_start(out=wqs[:, k, :], in_=wqr[:, k, :])
                nc.sync.dma_start(out=wks[:, k, :], in_=wkr[:, k, :])
            nc.sync.dma_start(out=ows[:], in_=owr)
            coss = cpool.tile([128, L], BF16, name="coss")
            nc.sync.dma_start(out=coss[:], in_=cosT2[:])
            sins = cpool.tile([128, L], BF16, name="sins")
            nc.sync.dma_start(out=sins[:], in_=sinT2[:])
            rms = cpool.tile([128, 128], BF16, name="rms")
            nc.sync.dma_start(out=rms[:], in_=rmat[:])
            ats = cpool.tile([128, 128], BF16, name="ats")
            nc.sync.dma_start(out=ats[:], in_=atri[:])
            ids = cpool.tile([128, 128], BF16, name="ids")
            nc.sync.dma_start(out=ids[:], in_=ident[:])

            with tc.tile_pool(name="persist", bufs=1) as ppool:
                qrope = ppool.tile([128, DT, L], BF16, name="qrope")
                krope = ppool.tile([128, DT, L], BF16, name="krope")
                # v in natural layout [tok%128, tok//128, head, hs+ones]
                vs = ppool.tile([128, 8, NH, 128], BF16, name="vs")
                # normalized attention output, transposed: [d%128, head-pair, tok]
                os2 = ppool.tile([128, DT, L], BF16, name="os2")

                nc.gpsimd.memset(vs[:, :, :, 64:128], 0.0)
                nc.gpsimd.memset(vs[:, :, :, 64:65], 1.0)
                ones64 = ppool.tile([65, 64], BF16, name="ones64")
                nc.gpsimd.memset(ones64[:], 1.0)

                # ---------------- Phase 1: projections + rope -------------
                with (
                    tc.tile_pool(name="psum", bufs=1, space="PSUM") as pp,
                    tc.tile_pool(name="work", bufs=1) as wp1,
                ):
                    wp2 = wp1
                    pp1 = pp2 = pp3 = pp
                    # v = x @ wv^T in natural layout
                    for bt in range(8):          # token tile of 128
                        for dh in range(NTT):    # d half of 512 (8 heads)
                            vp = pp1.tile([128, 512], F32, name="vp", tag="A", bufs=2)
                            for k in range(KT):
                                nc.tensor.matmul(
                                    vp[:],
                                    xs[:, k, bt * 128:(bt + 1) * 128],
                                    wvs[:, k, dh * 512:(dh + 1) * 512],
                                    start=(k == 0), stop=(k == KT - 1),
                                )
                            nc.scalar.copy(
                                out=vs[:, bt, dh * 8:(dh + 1) * 8, 0:64],
                                in_=vp[:].rearrange("p (h e) -> p h e", e=64),
                            )
                    # per head-pair group: qk proj + rope, then attention.
                    # The Z-bcast/recip/norm tail of group g is deferred until
                    # after group g+1's projection matmuls are queued, so the
                    # in-order PE never waits on it.
                    def emit_norm(items):
                        for (g_, c_, h_, stge_) in items:
                            zbp = pp2.tile([64, 512], F32, name="zbp",
                                           tag="ot0" if h_ == 0 else "ot1", bufs=1)
                            nc.tensor.matmul(zbp[:], ones64[64:65, :], stge_[64:65, :],
                                             start=True, stop=True)
                            rz = wp2.tile([64, 512], F32, name="rz", tag=f"rz{h_}", bufs=2)
                            nc.vector.reciprocal_approx_fast(out=rz[:], in_=zbp[:])
                            if debug and g_ == 0 and c_ == 0 and h_ == 0:
                                nc.sync.dma_start(out=dbg["rz"][:], in_=rz[:])
                            if h_ == 0:
                                nc.vector.tensor_mul(
                                    os2[0:64, g_, c_ * 512:(c_ + 1) * 512],
                                    stge_[0:64, :],
                                    rz[:],
                                )
                            else:
                                stg = wp2.tile([64, 512], BF16, name="stg", tag="stg", bufs=2)
                                nc.vector.tensor_mul(stg[:], stge_[0:64, :], rz[:])
                                nc.sync.dma_start(
                                    out=os2[64:128, g_, c_ * 512:(c_ + 1) * 512],
                                    in_=stg[:],
                                )

                    pending = []
                    for kd in range(DT):
                        for t in range(NTT):
                            for (ws, dest) in ((wqs, qrope), (wks, krope)):
                                qp = pp1.tile([128, 512], F32, name="qp", tag="qp", bufs=1)
                                for k in range(KT):
                                    nc.tensor.matmul(
                                        qp[:],
                                        ws[:, k, kd * 128:(kd + 1) * 128],
                                        xs[:, k, t * 512:(t + 1) * 512],
                                        start=(k == 0), stop=(k == KT - 1),
                                    )
                                qraw = wp1.tile([128, 512], BF16, name="qraw", tag="qraw", bufs=3)
                                nc.vector.tensor_copy(qraw[:], qp[:])
                                rp = pp1.tile([128, 512], F32, name="rp", tag="rp", bufs=1)
                                nc.tensor.matmul(rp[:], rms[:], qraw[:], start=True, stop=True)
                                m1 = wp1.tile([128, 512], BF16, name="m1", tag="m1", bufs=3)
                                nc.gpsimd.tensor_mul(m1[:], qraw[:], coss[:, t * 512:(t + 1) * 512])
                                m2 = wp1.tile([128, 512], BF16, name="m2", tag="m2", bufs=3)
                                nc.vector.tensor_mul(m2[:], rp[:], sins[:, t * 512:(t + 1) * 512])
                                nc.gpsimd.tensor_add(
                                    dest[:, kd, t * 512:(t + 1) * 512], m1[:], m2[:]
                                )

                        # deferred norm tail of the previous group
                        emit_norm(pending)
                        pending = []

                        # ---- attention for this head pair ----
                        g = kd
                        for c in range(NTT):     # i tile of 512
                            ot = [
                                pp2.tile([128, 512], F32, name=f"ot{h}_{g}_{c}",
                                         tag="ot0" if h == 0 else "ot1", bufs=1)
                                for h in range(2)
                            ]
                            nb = 4 * c + 4

                            def emit_omm(b_, es_, off_):
                                for h in range(2):
                                    nc.tensor.matmul(
                                        ot[h][0:128, off_:512],
                                        vs[:, b_, 2 * g + h, :],
                                        es_[:, h, off_:],
                                        start=(b_ == 0), stop=(b_ == nb - 1),
                                    )

                            prev_omm = None
                            for b in range(nb):  # j block of 128
                                scp = pp2.tile([128, 2, 512], F32, name="scp", tag="A", bufs=2)
                                partial = b >= 4 * c
                                off = max(0, 128 * b - 512 * c)
                                for h in range(2):
                                    nc.tensor.matmul(
                                        scp[:, h, off:],
                                        krope[64 * h:64 * h + 64, g, b * 128:(b + 1) * 128],
                                        qrope[64 * h:64 * h + 64, g, c * 512 + off:(c + 1) * 512],
                                        start=True, stop=not partial,
                                        tile_position=(64 * h, 0),
                                    )
                                if partial:
                                    for h in range(2):
                                        nc.tensor.matmul(
                                            scp[:, h, off:off + 128],
                                            ats[:], ids[:],
                                            start=False, stop=True,
                                        )
                                es = wp2.tile([128, 2, 512], BF16, name="es", tag="es", bufs=4)
                                nc.scalar.activation(
                                    out=es[:, :, off:], in_=scp[:, :, off:],
                                    func=AF.Exp, scale=0.125,
                                )
                                if debug and g == 0 and c == 0 and b == 0:
                                    dsc = ppool.tile([128, 2, 512], F32, name="dsc")
                                    nc.scalar.copy(out=dsc[:], in_=scp[:])
                                    nc.sync.dma_start(out=dbg["sc"][:], in_=dsc[:])
                                    nc.sync.dma_start(out=dbg["es"][:], in_=es[:])
                                if prev_omm is not None:
                                    emit_omm(*prev_omm)
                                prev_omm = (b, es, off)
                            emit_omm(*prev_omm)
                            # drain ot -> SBUF right away (releases the PSUM bank)
                            for h in range(2):
                                stge = wp2.tile([65, 512], BF16, name="stge", tag=f"stge{h}", bufs=2)
                                nc.vector.tensor_copy(stge[:], ot[h][0:65, :])
                                if debug and g == 0 and c == 0 and h == 0:
                                    dot = ppool.tile([65, 512], F32, name="dot")
                                    nc.scalar.copy(out=dot[:], in_=ot[0][0:65, :])
                                    nc.sync.dma_start(out=dbg["otraw"][:], in_=dot[:])
                                pending.append((g, c, h, stge))
                    emit_norm(pending)

                    # ------------- Phase 3: output projection -------------
                    wp3 = wp1
                    outr = outT.rearrange("(k p) t -> p k t", p=128)
                    for kh in range(DT):
                        for t in range(NTT):
                            fp = pp3.tile([128, 512], F32, name="fp", tag="A", bufs=2)
                            for k in range(KT):
                                nc.tensor.matmul(
                                    fp[:],
                                    ows[:, k, kh * 128:(kh + 1) * 128],
                                    os2[:, k, t * 512:(t + 1) * 512],
                                    start=(k == 0), stop=(k == KT - 1),
                                )
                            fo = wp3.tile([128, 512], BF16, name="fo", tag="fo", bufs=3)
                            nc.vector.tensor_copy(fo[:], fp[:])
                            nc.sync.dma_start(
                                out=outr[:, kh, t * 512:(t + 1) * 512], in_=fo[:]
                            )

                if debug:
                    nc.sync.dma_start(out=dbg["qrope"][:], in_=qrope[:])
                    nc.sync.dma_start(out=dbg["krope"][:], in_=krope[:])
                    nc.sync.dma_start(out=dbg["v"][:], in_=vs[:])
                    nc.sync.dma_start(out=dbg["o"][:], in_=os2[:])
    nc.finalize()
    return nc


def _host_consts():
    rmat = np.zeros((128, 128), np.float32)
    for m in range(128):
        if (m % 64) < 32:
            rmat[m + 32, m] = -1.0
        else:
            rmat[m - 32, m] = 1.0
    atri = -240.0 * np.triu(np.ones((128, 128), np.float32), 1)
    ident = np.eye(128, dtype=np.float32)
    return (rmat.astype(NPBF16), atri.astype(NPBF16), ident.astype(NPBF16))


def kernel(x, qw, kw, vw, ow, cos, sin, debug=False):
    x = np.asarray(x, np.float32)
    qw = np.asarray(qw, np.float32)
    kw = np.asarray(kw, np.float32)
    vw = np.asarray(vw, np.float32)
    ow = np.asarray(ow, np.float32)
    cos = np.asarray(cos, np.float32)
    sin = np.asarray(sin, np.float32)

    wqT = np.ascontiguousarray(qw.T).astype(NPBF16)
    wkT = np.ascontiguousarray(kw.T).astype(NPBF16)
    wvT = np.ascontiguousarray(vw.T).astype(NPBF16)
    owT = np.ascontiguousarray(ow.T).astype(NPBF16)
    cosT2 = np.concatenate([cos.T, cos.T], 0).astype(NPBF16)
    sinT2 = np.concatenate([sin.T, sin.T], 0).astype(NPBF16)
    rmat, atri, ident = _host_consts()

    shared = {
        "wqT": wqT, "wkT": wkT, "wvT": wvT, "owT": owT,
        "cosT2": np.ascontiguousarray(cosT2), "sinT2": np.ascontiguousarray(sinT2),
        "rmat": rmat, "atri": atri, "ident": ident,
    }
    in_maps = []
    for c in range(NCORES):
        b, p = divmod(c, P)
        xTc = np.ascontiguousarray(x[b, :, p, :].T).astype(NPBF16)
        m = dict(shared)
        m["xT"] = xTc
        in_maps.append(m)

    nc = build_nc(debug=debug)
    res = run_bass_kernel_spmd(nc, in_maps, list(range(NCORES)))

    y = np.empty((B, L, P, HID), np.float32)
    for c in range(NCORES):
        b, p = divmod(c, P)
        y[b, :, p, :] = np.asarray(res.results[c]["outT"], np.float32).T
    if debug:
        return y, res
    return y
